# revision 73
# baseline (speedup 1.0000x reference)
"""ViT attention block (B=64, N=197, H=12, hd=64, D=768) on 8 trn2 NeuronCores.

Pure data-parallel: 8 batches per core.  Single interleaved PE stream to keep
the HAM clock-gate warm (idle/low-utilization PE re-throttles to 1.2 GHz):

  prelude: q,k <- W_qk @ xT as 12 M=128 tiles (two heads per tile, k-outer
           loop over 4 live PSUM banks); v for all 8 batches.  xT is
           transposed on the HOST (no DMA-transpose, no xbar barrier).
  windows: attention for batch b interleaved with output-projection m-tiles
           (the N=512 proj matmuls keep the PE array streaming densely).

Attention per (batch b, head pair g, key chunk mc):
  S    = I @ rpb + k^T q      identity matmul (one N=394 strided-out MM)
                              preloads the relative-position bias into the
                              PSUM accumulation groups of both heads
  e2   = exp(S)               ACT, straight to bf16 SBUF (per-(b,g) tile)
  O    = [ones | v_h]^T @ e2  M=128: rows 0:64 = softmax sums (replicated),
                              rows 64:128 = unnormalized attention out
  B    = 1/O[0:64]            DVE reciprocal_approx_fast (input must sit at
                              partition base 0 - custom DVE ops ignore the
                              AP base_partition)
  outT = O[64:128] * B        DVE, head pairs stacked into partitions 0:64 /
                              64:128 of outT[128, 6, tok]
Projection: y = outT.T @ proj_wT (6 K=128 chunks) accumulated on top of a
K=1 ones-row matmul that preloads proj_b (v_bias folded in on host); evicted
by ACT copy, stored by gpsimd SWDGE.

Hardware constraints honored: PE operands and matmul PSUM outputs at
base_partition 0; one accumulation group per PSUM bank; gpsimd cannot access
PSUM; q pre-scaled by 1/8 on host.  DVE/ACT partition-base shifts verified
on HW.
"""

import os
import sys

import numpy as np

for _p in ("/opt/trn_rl_repo", os.path.expanduser("~/.axon_site/_ro/trn_rl_repo")):
    if os.path.isdir(_p) and _p not in sys.path:
        sys.path.insert(0, _p)

import ml_dtypes  # noqa: E402

B = 64
NTOK = 197
DIM = 768
HEADS = 12
HD = 64
NCORES = 8
BS = B // NCORES  # 8 batches per core
NT = BS * NTOK  # 1576 real tokens per core
NTP = 1600  # padded tokens (12x128 + 64)
SCALE = HD ** -0.5

_CACHE = {}


def _build_bass(stop_after=None):
    stop_after = stop_after or os.environ.get("K_STOP_AFTER", "")
    import concourse.mybir as mybir
    import concourse.tile as tile
    from concourse import bacc

    f32 = mybir.dt.float32
    bf16 = mybir.dt.bfloat16
    EXP = mybir.ActivationFunctionType.Exp

    nc = bacc.Bacc(
        "TRN2", target_bir_lowering=False, debug=False,
        num_devices=int(os.environ.get("K_NDEV", str(NCORES))),
    )

    xT_d = nc.dram_tensor("xT", [DIM, NTP], bf16, kind="ExternalInput")
    qkvw_d = nc.dram_tensor("qkv_wt", [DIM, 3 * DIM], bf16, kind="ExternalInput")
    qb_d = nc.dram_tensor("qb", [12, 64, 1], f32, kind="ExternalInput")
    projw_d = nc.dram_tensor("proj_wt", [DIM, DIM], bf16, kind="ExternalInput")
    pb_d = nc.dram_tensor("pb", [1, DIM], bf16, kind="ExternalInput")
    rpb_d = nc.dram_tensor("rpb", [2, 128, HEADS * NTOK], bf16, kind="ExternalInput")
    id_d = nc.dram_tensor("ident", [128, 128], bf16, kind="ExternalInput")
    y_d = nc.dram_tensor("y", [NT, DIM], f32, kind="ExternalOutput")

    NTILES = [(0, 512), (512, 512), (1024, 512), (1536, 64)]
    VTILES = [(0, 512), (512, 256)]

    with tile.TileContext(nc, linearize=bool(os.environ.get("K_LINEARIZE"))) as tc:
        with (
            tc.tile_pool(name="consts", bufs=1) as consts,
            tc.tile_pool(name="acts", bufs=1) as acts,
        ):
            # ---- constants ----
            projw = consts.tile([128, 6, DIM], bf16)  # head-pair K chunks
            rpb = consts.tile([128, 2, HEADS * NTOK], bf16)
            qb = consts.tile([64, 12, 1], f32)
            pb = consts.tile([1, DIM], bf16)
            ones1 = consts.tile([1, 128], bf16)
            ident = consts.tile([128, 128], bf16)

            # persistent activations.  qT2 slot h holds q_h in rows
            # (h%2)*64 and ZEROS in the other half; kT2 slot g holds the
            # head pair k_2g/k_2g+1 stacked.  S matmuls then contract over
            # K=128 (full PE-array activity - keeps the HAM clock-gate warm)
            qT2 = acts.tile([128, HEADS, NTP], bf16)
            kT2 = acts.tile([128, 6, NTP], bf16)
            vsb = acts.tile([128, 2 * BS, HEADS, 128], bf16)  # [ones | v_h]

            xp_cm = tc.tile_pool(name="xp", bufs=1)
            xp = xp_cm.__enter__()
            xt = xp.tile([128, 6, NTP], bf16)  # x transposed [c, tok]
            vw = xp.tile([128, 6, DIM], bf16)
            wqk_cm = tc.tile_pool(name="wqk", bufs=1)
            wqk = wqk_cm.__enter__()
            qkvw = wqk.tile([128, 6, 2 * DIM], bf16)

            qkvw_v = qkvw_d[:].rearrange("(k p) n -> p k n", p=128)
            xT_v = xT_d[:].rearrange("(k p) n -> p k n", p=128)
            for k in range(6):
                qa, qb_ = (nc.sync, nc.scalar) if k % 2 == 0 else (nc.scalar, nc.sync)
                qa.dma_start(out=qkvw[:, k, :], in_=qkvw_v[:, k, 0 : 2 * DIM])
                qb_.dma_start(out=xt[:, k, :], in_=xT_v[:, k, :])
            nc.scalar.dma_start(out=qb[:, :, :], in_=qb_d[:].rearrange("k p o -> p k o"))
            for k in range(6):
                nc.sync.dma_start(out=vw[:, k, :], in_=qkvw_v[:, k, 2 * DIM : 3 * DIM])
            for mc in range(2):
                nc.scalar.dma_start(out=rpb[:, mc, :], in_=rpb_d[mc, :, :])
            nc.scalar.dma_start(out=pb[:, :], in_=pb_d[:, :])
            nc.scalar.dma_start(out=ident[:, :], in_=id_d[:, :])
            projw_v = projw_d[:].rearrange("(k p) n -> p k n", p=128)
            for k in range(6):
                nc.scalar.dma_start(out=projw[:, k, :], in_=projw_v[:, k, :])
            nc.vector.memset(vsb[:, :, :, 0:64], 1.0)
            nc.vector.memset(ones1[:, :], 1.0)

            do_qkv = stop_after != "load"
            do_attn = do_qkv and stop_after != "qkv"
            do_proj = do_attn and stop_after != "attn"

            ps_v_cm = tc.tile_pool(name="ps_v", bufs=2, space="PSUM")
            ps_v = ps_v_cm.__enter__()
            ps_qk_cm = tc.tile_pool(name="ps_qk", bufs=1, space="PSUM")
            ps_qk = ps_qk_cm.__enter__()

            def emit_qk(t):
                pss = [
                    ps_qk.tile([128, 512], f32, name=f"pss{j}") for j in range(4)
                ]
                for k in range(6):
                    for j, (noff, nsz) in enumerate(NTILES):
                        nc.tensor.matmul(
                            pss[j][:, :nsz],
                            qkvw[:, k, t * 128 : (t + 1) * 128],
                            xt[:, k, noff : noff + nsz],
                            start=(k == 0),
                            stop=(k == 5),
                        )
                for j, (noff, nsz) in enumerate(NTILES):
                    if t < 6:  # q: add bias (pre-scaled on host)
                        nc.vector.tensor_scalar_add(
                            qT2[0:64, 2 * t, noff : noff + nsz],
                            pss[j][0:64, :nsz],
                            qb[:, 2 * t, 0:1],
                        )
                        nc.vector.tensor_scalar_add(
                            qT2[64:128, 2 * t + 1, noff : noff + nsz],
                            pss[j][64:128, :nsz],
                            qb[:, 2 * t + 1, 0:1],
                        )
                    else:  # k: single full-height copy (pair-stacked layout)
                        nc.scalar.copy(
                            kT2[:, t - 6, noff : noff + nsz],
                            pss[j][:, :nsz],
                        )

            def emit_v(b, mc, pool=None, pname="psv"):
                msz = 128 if mc == 0 else NTOK - 128
                toff = b * NTOK + mc * 128
                psv = (pool or ps_v).tile([128, DIM], f32, name=pname)
                for k in range(6):
                    for noff, nsz in VTILES:
                        nc.tensor.matmul(
                            psv[:msz, noff : noff + nsz],
                            xt[:, k, toff : toff + msz],
                            vw[:, k, noff : noff + nsz],
                            start=(k == 0),
                            stop=(k == 5),
                        )
                nc.scalar.copy(
                    vsb[:msz, b * 2 + mc, :, 64:128],
                    psv[:msz, :].rearrange("p (h d) -> p h d", d=64),
                )

            # ---- prelude: all of q,k and v(b0..b5) ----
            for t in range(12 if do_qkv else 0):
                emit_qk(t)
                if t == 11:
                    # zero the complementary q halves (needed before the
                    # K=128 S matmuls; disjoint from the eviction writes)
                    for h in range(HEADS):
                        r0 = 64 * (1 - h % 2)
                        nc.vector.memset(qT2[r0 : r0 + 64, h, :], 0.0)
            for b in range(6 if do_qkv else 0):
                for mc in range(2):
                    emit_v(b, mc)
            ps_qk_cm.__exit__(None, None, None)
            ps_v_cm.__exit__(None, None, None)
            wqk_cm.__exit__(None, None, None)

            otp_cm = tc.tile_pool(name="otp", bufs=1)
            otp = otp_cm.__enter__()
            outT = otp.tile([128, 6, NTP], bf16)  # attn out, head pairs stacked

            if stop_after == "qkv":
                nc.gpsimd.dma_start(out=y_d[0:64, :], in_=qT2[0:64, 0, 0:DIM])
                nc.gpsimd.dma_start(out=y_d[64:128, :], in_=qT2[64:128, 1, 0:DIM])
                nc.gpsimd.dma_start(
                    out=y_d[128:256, :].rearrange("p (h d) -> p h d", d=64),
                    in_=vsb[:, 0, :, 64:128],
                )
                nc.gpsimd.dma_start(out=y_d[256:320, :], in_=kT2[0:64, 0, 0:DIM])
                nc.gpsimd.dma_start(out=y_d[320:384, :], in_=kT2[64:128, 0, 0:DIM])
                nc.gpsimd.dma_start(
                    out=y_d[384:453, :].rearrange("p (h d) -> p h d", d=64),
                    in_=vsb[0:69, 1, :, 64:128],
                )

            # ---- attention interleaved with projection ----
            probes = {}
            if stop_after == "attn":
                probes["S"] = otp.tile([128, 2, NTOK], f32, name="probe_S")
                probes["e2"] = otp.tile([128, 2, NTOK], f32, name="probe_e2")
                probes["O"] = otp.tile([128, 2, NTOK], f32, name="probe_O")

            e2p_cm = tc.tile_pool(name="e2p", bufs=4)
            e2p = e2p_cm.__enter__()
            bp_cm = tc.tile_pool(name="bp", bufs=2)
            bp = bp_cm.__enter__()
            ps_s_cm = tc.tile_pool(name="ps_s", bufs=1, space="PSUM")
            ps_s = ps_s_cm.__enter__()
            ps_o_cm = tc.tile_pool(name="ps_o", bufs=1, space="PSUM")
            ps_o = ps_o_cm.__enter__()
            yp_cm = tc.tile_pool(name="yp", bufs=2)
            yp = yp_cm.__enter__()
            ps_y_cm = tc.tile_pool(name="ps_y", bufs=1, space="PSUM")
            ps_y = ps_y_cm.__enter__()

            DIV = mybir.AluOpType.divide
            NODIV = not os.environ.get("K_DIV")  # DVE divide rejected by BIR

            def emit_s(b, g, e2):
                """Scores (+rpb via identity matmul) + exp for one head pair
                (both key chunks in one 4-bank PSUM tile, one fused exp;
                the exp of the mc1 pad rows 69:128 is garbage, never read)."""
                tb = b * NTOK
                S = ps_s.tile([128, 2048], f32, name="S")
                for mc in range(2):
                    msz = 128 if mc == 0 else NTOK - 128
                    for hh in range(2):
                        h = 2 * g + hh
                        off = mc * 1024 + hh * 512
                        nc.tensor.matmul(
                            S[:msz, off : off + NTOK],
                            ident[:msz, :msz],
                            rpb[:msz, mc, h * NTOK : (h + 1) * NTOK],
                            start=True,
                            stop=False,
                        )
                        nc.tensor.matmul(
                            S[:msz, off : off + NTOK],
                            kT2[:, g, tb + mc * 128 : tb + mc * 128 + msz],
                            qT2[:, h, tb : tb + NTOK],
                            start=False,
                            stop=True,
                        )
                    # per-chunk exp: the mc0 half runs while the mc1 matmuls
                    # stream, so the S tile frees right after the last matmul
                    nc.scalar.activation(
                        e2[:msz, mc, :, :],
                        S[:msz, mc * 1024 : (mc + 1) * 1024]
                        .rearrange("p (s n) -> p s n", s=2)[:, :, :NTOK],
                        EXP,
                    )
                if probes and b == 0 and g == 0:
                    nc.vector.tensor_copy(
                        probes["S"][:, :, :],
                        S[:, 0:1024].rearrange("p (s n) -> p s n", s=2)[
                            :, :, :NTOK
                        ],
                    )
                    nc.scalar.copy(probes["e2"][:, :, :], e2[:, 0, :, :])

            def emit_o(b, g, e2, Bt):
                tb = b * NTOK
                O = ps_o.tile([128, 2, 512], f32, name="O")
                for hh in range(2):
                    h = 2 * g + hh
                    for mc in range(2):
                        msz = 128 if mc == 0 else NTOK - 128
                        nc.tensor.matmul(
                            O[:, hh, 0:NTOK],
                            vsb[:msz, b * 2 + mc, h, :],
                            e2[:msz, mc, hh, :],
                            start=(mc == 0),
                            stop=(mc == 1),
                        )
                if probes and b == 0 and g == 0:
                    nc.vector.tensor_copy(probes["O"][:, :, :], O[:, :, 0:NTOK])
                if NODIV:
                    nc.vector.reciprocal_approx_fast(
                        out=Bt[:, :, :], in_=O[0:64, :, 0:NTOK]
                    )
                    for hh in range(2):
                        nc.vector.tensor_mul(
                            outT[hh * 64 : (hh + 1) * 64, g, tb : tb + NTOK],
                            O[64:128, hh, 0:NTOK],
                            Bt[:, hh, :],
                        )
                else:
                    for hh in range(2):
                        nc.vector.tensor_tensor(
                            outT[hh * 64 : (hh + 1) * 64, g, tb : tb + NTOK],
                            O[64:128, hh, 0:NTOK],
                            O[0:64, hh, 0:NTOK],
                            DIV,
                        )

            def proj_steps(m):
                """Yield one proj m-tile as 8 small PE/evict steps so they can
                be woven between attention steps (keeps array duty high)."""
                moff = m * 128
                msz = min(128, NTP - moff)
                real = min(128, NT - moff)
                state = {}

                def s_pb():
                    state["Y"] = ps_y.tile([128, DIM], f32, name="Y")
                    for noff, nsz in VTILES:
                        nc.tensor.matmul(
                            state["Y"][:msz, noff : noff + nsz],
                            ones1[0:1, 0:msz],
                            pb[0:1, noff : noff + nsz],
                            start=True,
                            stop=False,
                        )

                yield s_pb
                for kp in range(6):
                    def s_kp(kp=kp):
                        for noff, nsz in VTILES:
                            nc.tensor.matmul(
                                state["Y"][:msz, noff : noff + nsz],
                                outT[:, kp, moff : moff + msz],
                                projw[:, kp, noff : noff + nsz],
                                start=False,
                                stop=(kp == 5),
                            )

                    yield s_kp

                def s_evict():
                    ysb = yp.tile([128, DIM], f32, name="ysb")
                    nc.scalar.copy(ysb[:msz, :], state["Y"][:msz, :])
                    nc.gpsimd.dma_start(
                        out=y_d[moff : moff + real, :], in_=ysb[:real, :]
                    )

                yield s_evict

            def emit_proj(m):
                for s in proj_steps(m):
                    s()

            # proj m-tile needs batches <= (128m+127)//197 fully emitted; a
            # batch's last O unit lands early in the NEXT window (WSTART
            # guards the weave).  v(b6,b7) fills the thin early windows.
            big_sched = {
                0: [("v", 6, 0), ("v", 6, 1)],
                1: [("v", 7, 0), ("v", 7, 1), ("p", 0)],
                2: [("p", 1)],
                3: [("p", 2), ("p", 3)],
                4: [("p", 4), ("p", 5)],
                5: [("p", 6)],
                6: [("p", 7), ("p", 8)],
                7: [("p", 9)],
            }
            tail_proj = [10, 11, 12]

            NB = int(os.environ.get("K_NB", str(BS)))
            OLAG = int(os.environ.get("K_OLAG", "2"))
            WSTART = 3  # first attn unit that proj steps may follow: by then
            # the previous batch's last O unit (and its normalizes) are emitted
            if do_attn:
                pending = []
                for b in range(NB):
                    bigs = list(big_sched.get(b, []))
                    if not do_proj:
                        bigs = [u for u in bigs if u[0] != "p"]
                    units = []
                    Bt = bp.tile([64, 2, NTOK], f32, name="Bt")
                    for g in range(6):
                        e2 = e2p.tile([128, 2, 2, NTOK], bf16, name="e2")
                        units.append((emit_s, (b, g, e2)))
                        pending.append((b, g, e2, Bt))
                        if len(pending) > OLAG:
                            units.append((emit_o, pending.pop(0)))
                    psteps = []
                    for u in bigs:
                        if u[0] == "v":
                            psteps.append(
                                lambda b_=u[1], mc_=u[2]: emit_v(
                                    b_, mc_, ps_y, "Y"
                                )
                            )
                        else:
                            psteps.extend(proj_steps(u[1]))
                    nslot = len(units) - WSTART
                    pi = 0
                    for i, u in enumerate(units):
                        u[0](*u[1])
                        if i >= WSTART:
                            want = ((i - WSTART + 1) * len(psteps)) // max(nslot, 1)
                            while pi < want:
                                psteps[pi]()
                                pi += 1
                    while pi < len(psteps):
                        psteps[pi]()
                        pi += 1
                for p in pending:
                    emit_o(p[0], p[1], p[2], p[3])
                if do_proj:
                    for m in tail_proj:
                        emit_proj(m)

            if stop_after == "attn":
                nc.gpsimd.dma_start(out=y_d[0:128, :], in_=outT[:, 0, 0:DIM])
                for nm, rows in (("S", (128, 256)), ("e2", (256, 384)),
                                 ("O", (384, 512))):
                    nc.gpsimd.dma_start(
                        out=y_d[rows[0] : rows[1], 0 : 2 * NTOK].rearrange(
                            "p (s n) -> p s n", s=2
                        ),
                        in_=probes[nm][:, :, :],
                    )

            for cm in (ps_y_cm, yp_cm, ps_o_cm, ps_s_cm, bp_cm, e2p_cm,
                       otp_cm, xp_cm):
                cm.__exit__(None, None, None)

    nc.compile()
    return nc


def _prep_inputs(x, qkv_w, q_bias, v_bias, rpb_table, proj_w, proj_b, rel_pos_index):
    bf16 = ml_dtypes.bfloat16
    x = np.asarray(x, np.float32)
    qkv_w = np.asarray(qkv_w, np.float32)
    q_bias = np.asarray(q_bias, np.float32)
    v_bias = np.asarray(v_bias, np.float32)
    rpb_table = np.asarray(rpb_table, np.float32)
    proj_w = np.asarray(proj_w, np.float32)
    proj_b = np.asarray(proj_b, np.float32)
    rel_pos_index = np.asarray(rel_pos_index)

    qkv_wt = qkv_w.T.copy()  # [768, 2304]
    qkv_wt[:, :DIM] *= SCALE
    qkv_wt = np.ascontiguousarray(qkv_wt, dtype=bf16)

    qb = (q_bias * SCALE).reshape(12, 64, 1).astype(np.float32)

    proj_wt = np.ascontiguousarray(proj_w.T, dtype=bf16)
    pb_eff = (proj_b + proj_w @ v_bias).reshape(1, DIM).astype(bf16)

    # bias[h, n, m] = rpb_table[rel_pos_index[n, m], h]; store raw (additive,
    # applied via identity matmul) as [m-chunk, m-in-chunk, h*197 + n]
    bias_nmh = rpb_table[rel_pos_index]  # [n, m, h]
    er = bias_nmh.transpose(1, 2, 0)  # [m, h, n]
    er = er.reshape(NTOK, HEADS * NTOK)
    er_pad = np.zeros((256, HEADS * NTOK), np.float32)
    er_pad[:NTOK] = er
    rpb = np.ascontiguousarray(er_pad.reshape(2, 128, HEADS * NTOK), dtype=bf16)

    shared = {
        "qkv_wt": qkv_wt,
        "qb": qb,
        "proj_wt": proj_wt,
        "pb": pb_eff,
        "rpb": rpb,
        "ident": np.eye(128, dtype=bf16),
    }
    in_maps = []
    for c in range(NCORES):
        xc = x[c * BS : (c + 1) * BS].reshape(NT, DIM)
        xp = np.zeros((NTP, DIM), np.float32)
        xp[:NT] = xc
        xT = np.ascontiguousarray(xp.T, dtype=bf16)  # [768, 1600]
        in_maps.append({"xT": xT, **shared})
    return in_maps


def run(inputs, trace=False):
    """Build (cached), run on 8 cores, return (y_full, BassKernelResults)."""
    from concourse.bass_utils import run_bass_kernel_spmd

    if "nc" not in _CACHE:
        _CACHE["nc"] = _build_bass()
    nc = _CACHE["nc"]
    in_maps = _prep_inputs(**{k: inputs[k] for k in (
        "x", "qkv_w", "q_bias", "v_bias", "rpb_table", "proj_w", "proj_b",
        "rel_pos_index")})
    try:
        res = run_bass_kernel_spmd(
            nc, in_maps, core_ids=list(range(NCORES)), trace=trace
        )
    except ModuleNotFoundError:
        # NTFF profile hook unavailable in this container; run untraced
        res = run_bass_kernel_spmd(
            nc, in_maps, core_ids=list(range(NCORES)), trace=False
        )
    y = np.concatenate(
        [res.results[c]["y"].reshape(BS, NTOK, DIM) for c in range(NCORES)], axis=0
    )
    return np.ascontiguousarray(y, np.float32), res


def kernel(**inputs) -> np.ndarray:
    y, _ = run(inputs, trace=False)
    return y


# revision 74
# speedup vs baseline: 1.4034x; 1.4034x over previous
"""ViT attention block (B=64, N=197, H=12, hd=64, D=768) on 8 trn2 NeuronCores.

Pure data-parallel: 8 batches per core.  Single interleaved PE stream to keep
the HAM clock-gate warm (idle/low-utilization PE re-throttles to 1.2 GHz):

  prelude: q,k <- W_qk @ xT as 12 M=128 tiles (two heads per tile, k-outer
           loop over 4 live PSUM banks); v for all 8 batches.  xT is
           transposed on the HOST (no DMA-transpose, no xbar barrier).
  windows: attention for batch b interleaved with output-projection m-tiles
           (the N=512 proj matmuls keep the PE array streaming densely).

Attention per (batch b, head pair g, key chunk mc):
  S    = I @ rpb + k^T q      identity matmul (one N=394 strided-out MM)
                              preloads the relative-position bias into the
                              PSUM accumulation groups of both heads
  e2   = exp(S)               ACT, straight to bf16 SBUF (per-(b,g) tile)
  O    = [ones | v_h]^T @ e2  M=128: rows 0:64 = softmax sums (replicated),
                              rows 64:128 = unnormalized attention out
  B    = 1/O[0:64]            DVE reciprocal_approx_fast (input must sit at
                              partition base 0 - custom DVE ops ignore the
                              AP base_partition)
  outT = O[64:128] * B        DVE, head pairs stacked into partitions 0:64 /
                              64:128 of outT[128, 6, tok]
Projection: y = outT.T @ proj_wT (6 K=128 chunks) accumulated on top of a
K=1 ones-row matmul that preloads proj_b (v_bias folded in on host); evicted
by ACT copy, stored by gpsimd SWDGE.

Hardware constraints honored: PE operands and matmul PSUM outputs at
base_partition 0; one accumulation group per PSUM bank; gpsimd cannot access
PSUM; q pre-scaled by 1/8 on host.  DVE/ACT partition-base shifts verified
on HW.
"""

import os
import sys

import numpy as np

for _p in ("/opt/trn_rl_repo", os.path.expanduser("~/.axon_site/_ro/trn_rl_repo")):
    if os.path.isdir(_p) and _p not in sys.path:
        sys.path.insert(0, _p)

import ml_dtypes  # noqa: E402

B = 64
NTOK = 197
DIM = 768
HEADS = 12
HD = 64
NCORES = 8
BS = B // NCORES  # 8 batches per core
NT = BS * NTOK  # 1576 real tokens per core
NTP = 1600  # padded tokens (12x128 + 64)
SCALE = HD ** -0.5

_CACHE = {}


def _build_bass(stop_after=None):
    stop_after = stop_after or os.environ.get("K_STOP_AFTER", "")
    import concourse.mybir as mybir
    import concourse.tile as tile
    from concourse import bacc

    f32 = mybir.dt.float32
    bf16 = mybir.dt.bfloat16
    EXP = mybir.ActivationFunctionType.Exp

    nc = bacc.Bacc(
        "TRN2", target_bir_lowering=False, debug=False,
        num_devices=int(os.environ.get("K_NDEV", str(NCORES))),
    )

    xT_d = nc.dram_tensor("xT", [DIM, NTP], bf16, kind="ExternalInput")
    qkvw_d = nc.dram_tensor("qkv_wt", [DIM, 3 * DIM], bf16, kind="ExternalInput")
    qb_d = nc.dram_tensor("qb", [12, 64, 1], f32, kind="ExternalInput")
    projw_d = nc.dram_tensor("proj_wt", [DIM, DIM], bf16, kind="ExternalInput")
    pb_d = nc.dram_tensor("pb", [1, DIM], bf16, kind="ExternalInput")
    rpb_d = nc.dram_tensor("rpb", [2, 128, HEADS * NTOK], bf16, kind="ExternalInput")
    id_d = nc.dram_tensor("ident", [128, 128], bf16, kind="ExternalInput")
    y_d = nc.dram_tensor("y", [NT, DIM], f32, kind="ExternalOutput")

    NTILES = [(0, 512), (512, 512), (1024, 512), (1536, 64)]
    VTILES = [(0, 512), (512, 256)]

    with tile.TileContext(nc, linearize=bool(os.environ.get("K_LINEARIZE"))) as tc:
        with (
            tc.tile_pool(name="consts", bufs=1) as consts,
            tc.tile_pool(name="acts", bufs=1) as acts,
        ):
            # ---- constants ----
            projw = consts.tile([128, 6, DIM], bf16)  # head-pair K chunks
            rpb = consts.tile([128, 2, HEADS * NTOK], bf16)
            qb = consts.tile([64, 12, 1], f32)
            pb = consts.tile([1, DIM], bf16)
            ones1 = consts.tile([1, 128], bf16)
            ident = consts.tile([128, 128], bf16)

            # persistent activations.  qT2 slot h holds q_h in rows
            # (h%2)*64 and ZEROS in the other half; kT2 slot g holds the
            # head pair k_2g/k_2g+1 stacked.  S matmuls then contract over
            # K=128 (full PE-array activity - keeps the HAM clock-gate warm)
            qT2 = acts.tile([128, HEADS, NTP], bf16)
            kT2 = acts.tile([128, 6, NTP], bf16)
            vsb = acts.tile([128, 2 * BS, HEADS, 128], bf16)  # [ones | v_h]

            xp_cm = tc.tile_pool(name="xp", bufs=1)
            xp = xp_cm.__enter__()
            xt = xp.tile([128, 6, NTP], bf16)  # x transposed [c, tok]
            vw = xp.tile([128, 6, DIM], bf16)
            wqk_cm = tc.tile_pool(name="wqk", bufs=1)
            wqk = wqk_cm.__enter__()
            qkvw = wqk.tile([128, 6, 2 * DIM], bf16)

            qkvw_v = qkvw_d[:].rearrange("(k p) n -> p k n", p=128)
            xT_v = xT_d[:].rearrange("(k p) n -> p k n", p=128)
            for k in range(6):
                qa, qb_ = (nc.sync, nc.scalar) if k % 2 == 0 else (nc.scalar, nc.sync)
                qa.dma_start(out=qkvw[:, k, :], in_=qkvw_v[:, k, 0 : 2 * DIM])
                qb_.dma_start(out=xt[:, k, :], in_=xT_v[:, k, :])
            nc.scalar.dma_start(out=qb[:, :, :], in_=qb_d[:].rearrange("k p o -> p k o"))
            for k in range(6):
                nc.sync.dma_start(out=vw[:, k, :], in_=qkvw_v[:, k, 2 * DIM : 3 * DIM])
            for mc in range(2):
                nc.scalar.dma_start(out=rpb[:, mc, :], in_=rpb_d[mc, :, :])
            nc.scalar.dma_start(out=pb[:, :], in_=pb_d[:, :])
            nc.scalar.dma_start(out=ident[:, :], in_=id_d[:, :])
            projw_v = projw_d[:].rearrange("(k p) n -> p k n", p=128)
            for k in range(6):
                nc.scalar.dma_start(out=projw[:, k, :], in_=projw_v[:, k, :])
            nc.vector.memset(vsb[:, :, :, 0:64], 1.0)
            nc.vector.memset(ones1[:, :], 1.0)

            do_qkv = stop_after != "load"
            do_attn = do_qkv and stop_after != "qkv"
            do_proj = do_attn and stop_after != "attn"

            ps_v_cm = tc.tile_pool(name="ps_v", bufs=2, space="PSUM")
            ps_v = ps_v_cm.__enter__()
            ps_qk_cm = tc.tile_pool(name="ps_qk", bufs=1, space="PSUM")
            ps_qk = ps_qk_cm.__enter__()

            def emit_qk(t):
                pss = [
                    ps_qk.tile([128, 512], f32, name=f"pss{j}") for j in range(4)
                ]
                for k in range(6):
                    for j, (noff, nsz) in enumerate(NTILES):
                        nc.tensor.matmul(
                            pss[j][:, :nsz],
                            qkvw[:, k, t * 128 : (t + 1) * 128],
                            xt[:, k, noff : noff + nsz],
                            start=(k == 0),
                            stop=(k == 5),
                        )
                for j, (noff, nsz) in enumerate(NTILES):
                    if t < 6:  # q: add bias (pre-scaled on host)
                        nc.vector.tensor_scalar_add(
                            qT2[0:64, 2 * t, noff : noff + nsz],
                            pss[j][0:64, :nsz],
                            qb[:, 2 * t, 0:1],
                        )
                        nc.vector.tensor_scalar_add(
                            qT2[64:128, 2 * t + 1, noff : noff + nsz],
                            pss[j][64:128, :nsz],
                            qb[:, 2 * t + 1, 0:1],
                        )
                    else:  # k: single full-height copy (pair-stacked layout)
                        nc.scalar.copy(
                            kT2[:, t - 6, noff : noff + nsz],
                            pss[j][:, :nsz],
                        )

            def emit_v(b, mc, pool=None, pname="psv"):
                msz = 128 if mc == 0 else NTOK - 128
                toff = b * NTOK + mc * 128
                psv = (pool or ps_v).tile([128, DIM], f32, name=pname)
                for k in range(6):
                    for noff, nsz in VTILES:
                        nc.tensor.matmul(
                            psv[:msz, noff : noff + nsz],
                            xt[:, k, toff : toff + msz],
                            vw[:, k, noff : noff + nsz],
                            start=(k == 0),
                            stop=(k == 5),
                        )
                nc.scalar.copy(
                    vsb[:msz, b * 2 + mc, :, 64:128],
                    psv[:msz, :].rearrange("p (h d) -> p h d", d=64),
                )

            # ---- prelude: all of q,k and v(b0..b5) ----
            for t in range(12 if do_qkv else 0):
                emit_qk(t)
                if t == 11:
                    # zero the complementary q halves (needed before the
                    # K=128 S matmuls; disjoint from the eviction writes)
                    for h in range(HEADS):
                        r0 = 64 * (1 - h % 2)
                        nc.vector.memset(qT2[r0 : r0 + 64, h, :], 0.0)
            for b in range(6 if do_qkv else 0):
                for mc in range(2):
                    emit_v(b, mc)
            ps_qk_cm.__exit__(None, None, None)
            ps_v_cm.__exit__(None, None, None)
            wqk_cm.__exit__(None, None, None)

            otp_cm = tc.tile_pool(name="otp", bufs=1)
            otp = otp_cm.__enter__()
            outT = otp.tile([128, 6, NTP], bf16)  # attn out, head pairs stacked

            if stop_after == "qkv":
                nc.gpsimd.dma_start(out=y_d[0:64, :], in_=qT2[0:64, 0, 0:DIM])
                nc.gpsimd.dma_start(out=y_d[64:128, :], in_=qT2[64:128, 1, 0:DIM])
                nc.gpsimd.dma_start(
                    out=y_d[128:256, :].rearrange("p (h d) -> p h d", d=64),
                    in_=vsb[:, 0, :, 64:128],
                )
                nc.gpsimd.dma_start(out=y_d[256:320, :], in_=kT2[0:64, 0, 0:DIM])
                nc.gpsimd.dma_start(out=y_d[320:384, :], in_=kT2[64:128, 0, 0:DIM])
                nc.gpsimd.dma_start(
                    out=y_d[384:453, :].rearrange("p (h d) -> p h d", d=64),
                    in_=vsb[0:69, 1, :, 64:128],
                )

            # ---- attention interleaved with projection ----
            probes = {}
            if stop_after == "attn":
                probes["S"] = otp.tile([128, 2, NTOK], f32, name="probe_S")
                probes["e2"] = otp.tile([128, 2, NTOK], f32, name="probe_e2")
                probes["O"] = otp.tile([128, 2, NTOK], f32, name="probe_O")

            e2p_cm = tc.tile_pool(name="e2p", bufs=4)
            e2p = e2p_cm.__enter__()
            bp_cm = tc.tile_pool(name="bp", bufs=2)
            bp = bp_cm.__enter__()
            ps_s_cm = tc.tile_pool(name="ps_s", bufs=1, space="PSUM")
            ps_s = ps_s_cm.__enter__()
            ps_o_cm = tc.tile_pool(name="ps_o", bufs=1, space="PSUM")
            ps_o = ps_o_cm.__enter__()
            yp_cm = tc.tile_pool(name="yp", bufs=2)
            yp = yp_cm.__enter__()
            ps_y_cm = tc.tile_pool(name="ps_y", bufs=1, space="PSUM")
            ps_y = ps_y_cm.__enter__()

            DIV = mybir.AluOpType.divide
            NODIV = not os.environ.get("K_DIV")  # DVE divide rejected by BIR

            def emit_s(b, g, e2):
                """Scores (+rpb via identity matmul) + exp for one head pair
                (both key chunks in one 4-bank PSUM tile, one fused exp;
                the exp of the mc1 pad rows 69:128 is garbage, never read)."""
                tb = b * NTOK
                S = ps_s.tile([128, 2048], f32, name="S")
                for mc in range(2):
                    msz = 128 if mc == 0 else NTOK - 128
                    for hh in range(2):
                        h = 2 * g + hh
                        off = mc * 1024 + hh * 512
                        nc.tensor.matmul(
                            S[:msz, off : off + NTOK],
                            ident[:msz, :msz],
                            rpb[:msz, mc, h * NTOK : (h + 1) * NTOK],
                            start=True,
                            stop=False,
                        )
                        nc.tensor.matmul(
                            S[:msz, off : off + NTOK],
                            kT2[:, g, tb + mc * 128 : tb + mc * 128 + msz],
                            qT2[:, h, tb : tb + NTOK],
                            start=False,
                            stop=True,
                        )
                Sv = S[:, :].rearrange("p (m s n) -> p m s n", m=2, s=2)[
                    :, :, :, :NTOK
                ]
                nc.scalar.activation(e2[:, :, :, :], Sv, EXP)
                if probes and b == 0 and g == 0:
                    nc.vector.tensor_copy(
                        probes["S"][:, :, :], Sv[:, 0, :, :]
                    )
                    nc.scalar.copy(probes["e2"][:, :, :], e2[:, 0, :, :])

            def emit_o(b, g, e2, Bt):
                tb = b * NTOK
                O = ps_o.tile([128, 2, 512], f32, name="O")
                for hh in range(2):
                    h = 2 * g + hh
                    for mc in range(2):
                        msz = 128 if mc == 0 else NTOK - 128
                        nc.tensor.matmul(
                            O[:, hh, 0:NTOK],
                            vsb[:msz, b * 2 + mc, h, :],
                            e2[:msz, mc, hh, :],
                            start=(mc == 0),
                            stop=(mc == 1),
                        )
                if probes and b == 0 and g == 0:
                    nc.vector.tensor_copy(probes["O"][:, :, :], O[:, :, 0:NTOK])
                if NODIV:
                    nc.vector.reciprocal_approx_fast(
                        out=Bt[:, :, :], in_=O[0:64, :, 0:NTOK]
                    )
                    for hh in range(2):
                        nc.vector.tensor_mul(
                            outT[hh * 64 : (hh + 1) * 64, g, tb : tb + NTOK],
                            O[64:128, hh, 0:NTOK],
                            Bt[:, hh, :],
                        )
                else:
                    for hh in range(2):
                        nc.vector.tensor_tensor(
                            outT[hh * 64 : (hh + 1) * 64, g, tb : tb + NTOK],
                            O[64:128, hh, 0:NTOK],
                            O[0:64, hh, 0:NTOK],
                            DIV,
                        )

            def proj_steps(m):
                """Yield one proj m-tile as 8 small PE/evict steps so they can
                be woven between attention steps (keeps array duty high)."""
                moff = m * 128
                msz = min(128, NTP - moff)
                real = min(128, NT - moff)
                state = {}

                def s_pb():
                    state["Y"] = ps_y.tile([128, DIM], f32, name="Y")
                    for noff, nsz in VTILES:
                        nc.tensor.matmul(
                            state["Y"][:msz, noff : noff + nsz],
                            ones1[0:1, 0:msz],
                            pb[0:1, noff : noff + nsz],
                            start=True,
                            stop=False,
                        )

                yield s_pb
                for kp in range(6):
                    def s_kp(kp=kp):
                        for noff, nsz in VTILES:
                            nc.tensor.matmul(
                                state["Y"][:msz, noff : noff + nsz],
                                outT[:, kp, moff : moff + msz],
                                projw[:, kp, noff : noff + nsz],
                                start=False,
                                stop=(kp == 5),
                            )

                    yield s_kp

                def s_evict():
                    ysb = yp.tile([128, DIM], f32, name="ysb")
                    nc.scalar.copy(ysb[:msz, :], state["Y"][:msz, :])
                    nc.gpsimd.dma_start(
                        out=y_d[moff : moff + real, :], in_=ysb[:real, :]
                    )

                yield s_evict

            def emit_proj(m):
                for s in proj_steps(m):
                    s()

            # proj m-tile needs batches <= (128m+127)//197 fully emitted; a
            # batch's last O unit lands early in the NEXT window (WSTART
            # guards the weave).  v(b6,b7) fills the thin early windows.
            big_sched = {
                0: [("v", 6, 0), ("v", 6, 1)],
                1: [("v", 7, 0), ("v", 7, 1), ("p", 0)],
                2: [("p", 1)],
                3: [("p", 2), ("p", 3)],
                4: [("p", 4), ("p", 5)],
                5: [("p", 6)],
                6: [("p", 7), ("p", 8)],
                7: [("p", 9)],
            }
            tail_proj = [10, 11, 12]

            NB = int(os.environ.get("K_NB", str(BS)))
            OLAG = int(os.environ.get("K_OLAG", "2"))
            WSTART = 3  # first attn unit that proj steps may follow: by then
            # the previous batch's last O unit (and its normalizes) are emitted
            if do_attn:
                pending = []
                for b in range(NB):
                    bigs = list(big_sched.get(b, []))
                    if not do_proj:
                        bigs = [u for u in bigs if u[0] != "p"]
                    units = []
                    Bt = bp.tile([64, 2, NTOK], f32, name="Bt")
                    for g in range(6):
                        e2 = e2p.tile([128, 2, 2, NTOK], bf16, name="e2")
                        units.append((emit_s, (b, g, e2)))
                        pending.append((b, g, e2, Bt))
                        if len(pending) > OLAG:
                            units.append((emit_o, pending.pop(0)))
                    psteps = []
                    for u in bigs:
                        if u[0] == "v":
                            psteps.append(
                                lambda b_=u[1], mc_=u[2]: emit_v(
                                    b_, mc_, ps_y, "Y"
                                )
                            )
                        else:
                            psteps.extend(proj_steps(u[1]))
                    nslot = len(units) - WSTART
                    pi = 0
                    for i, u in enumerate(units):
                        u[0](*u[1])
                        if i >= WSTART:
                            want = ((i - WSTART + 1) * len(psteps)) // max(nslot, 1)
                            while pi < want:
                                psteps[pi]()
                                pi += 1
                    while pi < len(psteps):
                        psteps[pi]()
                        pi += 1
                for p in pending:
                    emit_o(p[0], p[1], p[2], p[3])
                if do_proj:
                    for m in tail_proj:
                        emit_proj(m)

            if stop_after == "attn":
                nc.gpsimd.dma_start(out=y_d[0:128, :], in_=outT[:, 0, 0:DIM])
                for nm, rows in (("S", (128, 256)), ("e2", (256, 384)),
                                 ("O", (384, 512))):
                    nc.gpsimd.dma_start(
                        out=y_d[rows[0] : rows[1], 0 : 2 * NTOK].rearrange(
                            "p (s n) -> p s n", s=2
                        ),
                        in_=probes[nm][:, :, :],
                    )

            for cm in (ps_y_cm, yp_cm, ps_o_cm, ps_s_cm, bp_cm, e2p_cm,
                       otp_cm, xp_cm):
                cm.__exit__(None, None, None)

    nc.compile()
    return nc


def _prep_inputs(x, qkv_w, q_bias, v_bias, rpb_table, proj_w, proj_b, rel_pos_index):
    bf16 = ml_dtypes.bfloat16
    x = np.asarray(x, np.float32)
    qkv_w = np.asarray(qkv_w, np.float32)
    q_bias = np.asarray(q_bias, np.float32)
    v_bias = np.asarray(v_bias, np.float32)
    rpb_table = np.asarray(rpb_table, np.float32)
    proj_w = np.asarray(proj_w, np.float32)
    proj_b = np.asarray(proj_b, np.float32)
    rel_pos_index = np.asarray(rel_pos_index)

    qkv_wt = qkv_w.T.copy()  # [768, 2304]
    qkv_wt[:, :DIM] *= SCALE
    qkv_wt = np.ascontiguousarray(qkv_wt, dtype=bf16)

    qb = (q_bias * SCALE).reshape(12, 64, 1).astype(np.float32)

    proj_wt = np.ascontiguousarray(proj_w.T, dtype=bf16)
    pb_eff = (proj_b + proj_w @ v_bias).reshape(1, DIM).astype(bf16)

    # bias[h, n, m] = rpb_table[rel_pos_index[n, m], h]; store raw (additive,
    # applied via identity matmul) as [m-chunk, m-in-chunk, h*197 + n]
    bias_nmh = rpb_table[rel_pos_index]  # [n, m, h]
    er = bias_nmh.transpose(1, 2, 0)  # [m, h, n]
    er = er.reshape(NTOK, HEADS * NTOK)
    er_pad = np.zeros((256, HEADS * NTOK), np.float32)
    er_pad[:NTOK] = er
    rpb = np.ascontiguousarray(er_pad.reshape(2, 128, HEADS * NTOK), dtype=bf16)

    shared = {
        "qkv_wt": qkv_wt,
        "qb": qb,
        "proj_wt": proj_wt,
        "pb": pb_eff,
        "rpb": rpb,
        "ident": np.eye(128, dtype=bf16),
    }
    in_maps = []
    for c in range(NCORES):
        xc = x[c * BS : (c + 1) * BS].reshape(NT, DIM)
        xp = np.zeros((NTP, DIM), np.float32)
        xp[:NT] = xc
        xT = np.ascontiguousarray(xp.T, dtype=bf16)  # [768, 1600]
        in_maps.append({"xT": xT, **shared})
    return in_maps


def run(inputs, trace=False):
    """Build (cached), run on 8 cores, return (y_full, BassKernelResults)."""
    from concourse.bass_utils import run_bass_kernel_spmd

    if "nc" not in _CACHE:
        _CACHE["nc"] = _build_bass()
    nc = _CACHE["nc"]
    in_maps = _prep_inputs(**{k: inputs[k] for k in (
        "x", "qkv_w", "q_bias", "v_bias", "rpb_table", "proj_w", "proj_b",
        "rel_pos_index")})
    try:
        res = run_bass_kernel_spmd(
            nc, in_maps, core_ids=list(range(NCORES)), trace=trace
        )
    except ModuleNotFoundError:
        # NTFF profile hook unavailable in this container; run untraced
        res = run_bass_kernel_spmd(
            nc, in_maps, core_ids=list(range(NCORES)), trace=False
        )
    y = np.concatenate(
        [res.results[c]["y"].reshape(BS, NTOK, DIM) for c in range(NCORES)], axis=0
    )
    return np.ascontiguousarray(y, np.float32), res


def kernel(**inputs) -> np.ndarray:
    y, _ = run(inputs, trace=False)
    return y
